# revision 4
# baseline (speedup 1.0000x reference)
"""Trainium2 Bass kernel for nn_CPABActivationDifferent.

The reference applies, per channel c, a scalar map G_c to every element of
x[:, c] (the sort/unsort in the reference is a mathematical no-op: argsort +
elementwise map + inverse argsort == elementwise map). G_c is the numeric
CPAB integrator, which is piecewise-affine in x. We fit, on the host, a
per-channel continuous piecewise-linear approximation in relu basis

    y = x + sum_j dal[c,j] * relu(x - s[c,j])

(identity outside [-3, 3] is exact: the reference maps out-of-domain points
to themselves), and evaluate it on device with one custom Vector-engine
instruction per knot: acc <- acc + relu(x - s_j) * dal_j, with s_j/dal_j as
per-partition scalars. Data is processed as [channel, node] tiles (TensorE
transposes on the way in/out), sharded over the node dimension across the
8 NeuronCores.
"""

import os
import numpy as np

# ---------------------------------------------------------------- reference
# fixed problem constants (from the nn.Module / spec)
NC = 16
RADIUS = 3.0
NSTEPS1, NSTEPS2 = 10, 5
N_FULL, C_FULL = 131072, 256
N_CORES = 8
N_SHARD = N_FULL // N_CORES

K_KNOTS = int(os.environ.get("CPAB_K", "64"))
F_BLK = 2048  # nodes per chain block (free-dim length of DVE instructions)


def _make_basis(nc: int) -> np.ndarray:
    L = np.zeros((nc + 1, 2 * nc), dtype=np.float64)
    for k in range(1, nc):
        xk = k / nc
        L[k - 1, 2 * (k - 1)] = xk
        L[k - 1, 2 * (k - 1) + 1] = 1.0
        L[k - 1, 2 * k] = -xk
        L[k - 1, 2 * k + 1] = -1.0
    L[nc - 1, 1] = 1.0
    L[nc, 2 * (nc - 1)] = 1.0
    L[nc, 2 * nc - 1] = 1.0
    _, _, Vt = np.linalg.svd(L)
    return np.ascontiguousarray(Vt[nc + 1:].T)


_B32 = _make_basis(NC).astype(np.float32)


def _ab_from_theta(theta: np.ndarray):
    A = (theta.astype(np.float32) @ _B32.T).astype(np.float32)
    return A[:, 0::2], A[:, 1::2]


def _integrate_multi(phi, a, b, time, dtype):
    dt = dtype(dtype(time) / NSTEPS1)
    ddt = dtype(dt / NSTEPS2)
    a = a.astype(dtype)
    b = b.astype(dtype)
    phi = phi.astype(dtype)
    rows = np.arange(a.shape[0])[:, None]

    def cell(v):
        return np.clip(np.floor(v * dtype(NC)).astype(np.int32), 0, NC - 1)

    eta_tab = np.exp(dt * a).astype(dtype)
    nz = np.abs(a) > 1e-7
    safe_a = np.where(nz, a, dtype(1.0)).astype(dtype)

    for _ in range(NSTEPS1):
        c0 = cell(phi)
        ac = a[rows, c0]
        bc = b[rows, c0]
        eta = eta_tab[rows, c0]
        sa = safe_a[rows, c0]
        phi_cf = np.where(nz[rows, c0],
                          (eta * phi + (bc / sa) * (eta - dtype(1.0))).astype(dtype),
                          (phi + bc * dt).astype(dtype))
        stay = cell(phi_cf) == c0
        p = phi
        for _ in range(NSTEPS2):
            cc = cell(p)
            v = (a[rows, cc] * p + b[rows, cc]).astype(dtype)
            p = (p + ddt * v).astype(dtype)
        phi = np.where(stay, phi_cf, p).astype(dtype)
    return phi


def _G_multi(x, a, b, time, dtype=np.float64):
    C = a.shape[0]
    x = np.asarray(x, dtype)
    if x.ndim == 1:
        x = np.broadcast_to(x[None, :], (C, x.shape[0]))
    xs = ((x + dtype(RADIUS)) / dtype(2 * RADIUS)).astype(dtype)
    ood = (xs >= dtype(1.0)) | (xs <= dtype(0.0))
    xt = (_integrate_multi(xs, a, b, time, dtype) * dtype(2 * RADIUS)
          - dtype(RADIUS)).astype(dtype)
    return np.where(ood, x, xt).astype(dtype)


# ------------------------------------------------------------------ fitting
def _fit_tables(theta: np.ndarray, time, K: int):
    """Returns s [C, K] f32 knot positions and dal [C, K] f32 relu coeffs."""
    a, b = _ab_from_theta(theta)
    C = a.shape[0]
    M = 1 << 15
    xg = np.linspace(-RADIUS, RADIUS, M + 1)
    Y = _G_multi(xg, a, b, time, np.float64)
    h = xg[1] - xg[0]

    # start knots: curvature equidistribution
    K_start = max(8, int(K * 0.35))
    d2 = np.abs(np.diff(Y, 2, axis=1))
    w = np.sqrt(d2 + 1e-14)
    cw = np.cumsum(w, axis=1)
    cw = cw / cw[:, -1:]
    q = np.linspace(0, 1, K_start)[1:-1]

    knots = np.empty((C, K))
    vals = np.empty((C, K))
    for c in range(C):
        idx = np.searchsorted(cw[c], q)
        kn = np.unique(np.concatenate([[-RADIUS, RADIUS], xg[1 + idx]]))
        # greedy insertion at worst-error point
        while len(kn) < K:
            yk = np.interp(kn, xg, Y[c])
            approx = np.interp(xg, kn, yk)
            err = np.abs(approx - Y[c])
            iw = int(np.argmax(err))
            xw = xg[iw]
            if np.min(np.abs(kn - xw)) < 0.5 * h:
                err[max(0, iw - 1):iw + 2] = 0
                iw = int(np.argmax(err))
                xw = xg[iw]
                if np.min(np.abs(kn - xw)) < 0.5 * h:
                    kn = np.sort(np.append(kn, 0.5 * (kn[0] + kn[1])))
                    continue
            kn = np.sort(np.append(kn, xw))
        knots[c] = kn
        vals[c] = np.interp(kn, xg, Y[c])

    # minimax polish of knot values
    for _ in range(8):
        for c in range(C):
            approx = np.interp(xg, knots[c], vals[c])
            e = Y[c] - approx
            starts = np.searchsorted(xg, knots[c])[:-1]
            pmax = np.maximum.reduceat(e, starts)
            pmin = np.minimum.reduceat(e, starts)
            mid = 0.5 * (pmax + pmin)
            adj = np.empty(K)
            adj[0] = 0.0
            adj[-1] = 0.0
            adj[1:-1] = 0.5 * (mid[:-1] + mid[1:])
            vals[c] += 0.7 * adj

    # convert to relu coefficients (identity base outside [-3, 3])
    slopes = np.empty((C, K + 1))
    slopes[:, 0] = 1.0
    slopes[:, 1:-1] = np.diff(vals, axis=1) / np.diff(knots, axis=1)
    slopes[:, -1] = 1.0
    dal = np.diff(slopes, axis=1)
    return knots.astype(np.float32), dal.astype(np.float32)


# ------------------------------------------------------------- device setup
_PWL_OP = None


def _register_pwl_op():
    global _PWL_OP
    if _PWL_OP is not None:
        return _PWL_OP
    from concourse import dve_ops
    from concourse.dve_spec import Spec, Src0, Src1, C0, C1, relu, lower
    from concourse.dve_uop import DveOpSpec

    name = "PWL_STEP_ANT"
    for op in dve_ops.OPS:
        if op.name == name:
            _PWL_OP = op
            return op
    spec = Spec(
        body=Src0 + relu(Src1 - C0) * C1,
        reference=lambda in0, in1, s0, s1, imm2: (
            in0 + np.maximum(in1 - s0, np.float32(0.0)).astype(np.float32) * s1
        ),
    )
    opcode = dve_ops._CUSTOM_DVE_ROW_BASE + len(dve_ops.OPS)
    shas = {}
    for ver in ("v3", "v4"):
        uops = lower(spec, ver=ver)
        shas[ver] = DveOpSpec(name=name, opcode=opcode, uops=uops,
                              rd1_en=True).sha(ver)
    op = dve_ops.DveOp(name, spec, subdim=False, uops_sha=shas)
    dve_ops.OPS.append(op)
    dve_ops._SUB_OPCODE_FOR_NAME[name] = opcode
    dve_ops.CUSTOM_DVE_SPECS[name] = spec
    _PWL_OP = op
    return op


_PROG_CACHE = {}


def _build_program(n_shard: int, K: int):
    key = (n_shard, K)
    if key in _PROG_CACHE:
        return _PROG_CACHE[key]

    import concourse.bacc as bacc
    import concourse.mybir as mybir
    import concourse.tile as tile
    from concourse import masks

    op = _register_pwl_op()
    f32 = mybir.dt.float32
    NGRP = C_FULL // 128  # channel groups of 128 partitions
    NT = F_BLK // 128     # node-tiles per block
    NBLK = n_shard // F_BLK

    nc = bacc.Bacc("TRN2", target_bir_lowering=False, debug=False,
                   num_devices=N_CORES)
    x_in = nc.dram_tensor("x_in", [n_shard, C_FULL], f32,
                          kind="ExternalInput").ap()
    s_in = nc.dram_tensor("s_tab", [NGRP, 128, K], f32,
                          kind="ExternalInput").ap()
    d_in = nc.dram_tensor("d_tab", [NGRP, 128, K], f32,
                          kind="ExternalInput").ap()
    y_out = nc.dram_tensor("y_out", [n_shard, C_FULL], f32,
                           kind="ExternalOutput").ap()

    with tile.TileContext(nc) as tc:
        with (
            tc.tile_pool(name="const", bufs=1) as const_pool,
            tc.tile_pool(name="io", bufs=4) as io_pool,
            tc.tile_pool(name="big", bufs=2) as big_pool,
            tc.tile_pool(name="ps", bufs=4, space="PSUM") as psum_pool,
        ):
            ident = const_pool.tile([128, 128], f32, tag="ident", name="ident")
            masks.make_identity(nc, ident[:])
            s_t = []
            d_t = []
            for g in range(NGRP):
                st = const_pool.tile([128, K], f32, tag=f"s{g}", name=f"s{g}")
                dt_ = const_pool.tile([128, K], f32, tag=f"d{g}", name=f"d{g}")
                nc.sync.dma_start(st[:], s_in[g])
                nc.sync.dma_start(dt_[:], d_in[g])
                s_t.append(st)
                d_t.append(dt_)

            for blk in range(NBLK):
                xT = [big_pool.tile([128, F_BLK], f32, tag=f"xT{g}", name=f"xT{g}")
                      for g in range(NGRP)]
                acc = [big_pool.tile([128, F_BLK], f32, tag=f"acc{g}", name=f"acc{g}")
                       for g in range(NGRP)]
                row0 = blk * F_BLK
                for t in range(NT):
                    r = row0 + t * 128
                    xin = io_pool.tile([128, C_FULL], f32, tag="xin", name="xin")
                    nc.sync.dma_start(xin[:], x_in[r:r + 128, :])
                    for g in range(NGRP):
                        ps = psum_pool.tile([128, 128], f32, tag="ps", name="ps")
                        nc.tensor.transpose(
                            ps[:], xin[:, g * 128:(g + 1) * 128], ident[:])
                        nc.scalar.copy(xT[g][:, t * 128:(t + 1) * 128], ps[:])
                for g in range(NGRP):
                    # chain init: acc = x (identity base)
                    nc.vector.tensor_copy(acc[g][:], xT[g][:])
                # interleave the groups' chains so consecutive DVE
                # instructions are independent (hides the pipe drain)
                for j in range(K):
                    for g in range(NGRP):
                        nc.vector._custom_dve(
                            op, out=acc[g][:], in0=acc[g][:], in1=xT[g][:],
                            s0=s_t[g][:, j:j + 1], s1=d_t[g][:, j:j + 1])
                for t in range(NT):
                    r = row0 + t * 128
                    yout = io_pool.tile([128, C_FULL], f32, tag="yout", name="yout")
                    for g in range(NGRP):
                        ps = psum_pool.tile([128, 128], f32, tag="ps", name="ps")
                        nc.tensor.transpose(
                            ps[:], acc[g][:, t * 128:(t + 1) * 128], ident[:])
                        nc.scalar.copy(yout[:, g * 128:(g + 1) * 128], ps[:])
                    nc.sync.dma_start(y_out[r:r + 128, :], yout[:])

    nc.compile()
    _PROG_CACHE[key] = nc
    return nc


LAST_EXEC_NS = None
LAST_RESULTS = None


def _prepare(x, theta, t_val):
    """Fit tables + build/compile the program. Returns (nc, in_maps)."""
    x = np.ascontiguousarray(np.asarray(x, dtype=np.float32))
    n, c = x.shape
    assert c == C_FULL and n % N_CORES == 0
    n_shard = n // N_CORES

    s, dal = _fit_tables(np.asarray(theta, np.float32), t_val, K_KNOTS)
    ngrp = c // 128
    s_tab = np.ascontiguousarray(s.reshape(ngrp, 128, K_KNOTS))
    d_tab = np.ascontiguousarray(dal.reshape(ngrp, 128, K_KNOTS))

    nc = _build_program(n_shard, K_KNOTS)

    in_maps = []
    for i in range(N_CORES):
        in_maps.append({
            "x_in": np.ascontiguousarray(x[i * n_shard:(i + 1) * n_shard]),
            "s_tab": s_tab,
            "d_tab": d_tab,
        })
    return nc, in_maps


def _execute(nc, in_maps):
    global LAST_EXEC_NS, LAST_RESULTS
    from concourse import bass_utils

    trace = bool(int(os.environ.get("CPAB_TRACE", "0")))
    res = bass_utils.run_bass_kernel_spmd(
        nc, in_maps, core_ids=list(range(N_CORES)), trace=trace)
    LAST_EXEC_NS = res.exec_time_ns
    LAST_RESULTS = res
    return np.concatenate([r["y_out"] for r in res.results], axis=0)


def kernel(x, edge_index, edge_attr, batch, time, theta):
    theta_np = np.asarray(theta)
    t_val = np.asarray(time).reshape(()).item()
    nc, in_maps = _prepare(x, theta_np, t_val)
    out = _execute(nc, in_maps)
    return (out, theta_np)


if __name__ == "__main__":
    rng = np.random.default_rng(0)
    x = rng.standard_normal((N_FULL, C_FULL)).astype(np.float32)
    theta = (0.5 * rng.standard_normal((C_FULL, NC - 1))).astype(np.float32)
    out, th = kernel(x, None, None, None, 1, theta)
    print("out", out.shape, out.dtype, "exec_ns", LAST_EXEC_NS)
